# revision 4
# baseline (speedup 1.0000x reference)
"""Trainium2 Bass kernel for GQA attention (B=2, S=2048, D=2048, 16 q-heads,
4 kv-heads, head_dim=128, RoPE, causal) sharded over 8 NeuronCores.

Sharding: core c handles batch b = c//4 and q-head group g = c%4
(q-heads 4g..4g+3, which share kv-head g).  Each core computes a partial
output o_part[b] = sum_{its heads} attn_head @ Wo_head; the host sums the
4 partials per batch.

Schedule: fully software-pipelined over 512-row blocks.  Per block sb:
q/k/v projections (rope fused) -> attention heads 0..3 -> O-projection of
block sb-1 interleaved between heads.  x arrives block-major on the sync
HWDGE ring (contiguous 1KB+ segments), weights on the scalar ring, so the
PE starts within ~3us and never waits for a monolithic load.
"""

import sys

sys.path.insert(0, "/opt/trn_rl_repo")

import math

import ml_dtypes
import numpy as np

P = 128
NEG = -1.0e9
EXP_BIAS = -8.0  # exp(s - 8): cancels in softmax normalization, avoids overflow


def build_nc(S=2048, D=2048, QH=4, H=128, theta=10000.0):
    """Build the per-core Bass graph.

    Per-core problem: xt [NSB, D, SB] bf16 (x.T in 512-col blocks),
    positions [S] i32, wq [QH, P, DK, H] bf16 (pre-scaled by 1/sqrt(H),
    d-chunk-shuffled), wk/wv [P, DK, H] bf16, wo [QH, H, D] bf16
    ->  o [S, D] bf16 (partial over heads).
    """
    import concourse.bacc as bacc
    import concourse.mybir as mybir
    from concourse import tile
    from concourse.masks import make_identity

    f32 = mybir.dt.float32
    bf16 = mybir.dt.bfloat16
    i32 = mybir.dt.int32
    ADD = mybir.AluOpType.add
    MULT = mybir.AluOpType.mult
    EXP = mybir.ActivationFunctionType.Exp
    SIN = mybir.ActivationFunctionType.Sin

    assert H == P
    HH = H // 2  # 64
    DK = D // P  # d-chunks (16)
    SB = min(512, S)  # block width
    NSB = S // SB  # blocks (4)
    RB = SB // P  # 128-subtiles per block (4)
    NT = S // P  # t tiles (16)
    CS = min(512, S)  # rope table chunk width
    NCS = S // CS

    nc = bacc.Bacc(None, target_bir_lowering=False)

    xt_d = nc.declare_dram_parameter("xt", [NSB, D, SB], bf16, isOutput=False)
    pos_d = nc.declare_dram_parameter("positions", [S], i32, isOutput=False)
    wq_d = nc.declare_dram_parameter("wq", [QH, P, DK, H], bf16, isOutput=False)
    wk_d = nc.declare_dram_parameter("wk", [P, DK, H], bf16, isOutput=False)
    wv_d = nc.declare_dram_parameter("wv", [P, DK, H], bf16, isOutput=False)
    wo_d = nc.declare_dram_parameter("wo", [QH, H, D], bf16, isOutput=False)
    o_d = nc.declare_dram_parameter("o", [S, D], bf16, isOutput=True)

    from contextlib import ExitStack

    with tile.TileContext(nc) as tc, ExitStack() as es:
        # ---------------- pools ----------------
        const = es.enter_context(tc.tile_pool(name="const", bufs=1))
        persist = es.enter_context(tc.tile_pool(name="persist", bufs=1))
        small = es.enter_context(tc.tile_pool(name="small", bufs=2))
        rope_p = es.enter_context(tc.tile_pool(name="rope", bufs=1))
        pt_pool = es.enter_context(tc.tile_pool(name="pt", bufs=8))
        ob_pool = es.enter_context(tc.tile_pool(name="ob", bufs=2))
        at_pool = es.enter_context(tc.tile_pool(name="at", bufs=1))
        ps_sc = es.enter_context(tc.tile_pool(name="ps_sc", bufs=4, space="PSUM"))
        ps_av = es.enter_context(tc.tile_pool(name="ps_av", bufs=1, space="PSUM"))

        # ---------------- DMA issue: weights on the scalar HWDGE ring ------
        # (sync ring carries x + outputs; scalar engine is idle at start so
        # the serialized trigger-transfer time is hidden there)
        wq_sb = []
        for h in range(QH):
            wb = persist.tile([P, DK, H], bf16, name=f"wq{h}", tag=f"wq{h}")
            nc.scalar.dma_start(wb[:], wq_d[h])
            wq_sb.append(wb)
        wk_sb = persist.tile([P, DK, H], bf16, name="wk", tag="wk")
        nc.scalar.dma_start(wk_sb[:], wk_d[:])
        wv_sb = persist.tile([P, DK, H], bf16, name="wv", tag="wv")
        nc.scalar.dma_start(wv_sb[:], wv_d[:])
        wo_sb = []
        for h in range(QH):
            wb = persist.tile([P, D], bf16, name=f"wo{h}", tag=f"wo{h}")
            nc.scalar.dma_start(wb[:], wo_d[h])
            wo_sb.append(wb)

        # ---------------- x: block-major on the sync ring ----------------
        # xT layout [p, dk, s]: element x(s, d = dk*P + p).  Block sb's
        # slice is a contiguous 2MB read; written 1KB-contiguous per
        # partition.  Block 0 is split into 4 dk-groups so the first
        # projection matmuls start after ~0.5MB instead of 2MB.
        xT = persist.tile([P, DK, S], bf16)
        DG = 4  # dk-group size for block 0
        for dk0 in range(0, DK, DG):
            nc.sync.dma_start(
                xT[:, dk0 : dk0 + DG, 0:SB],
                xt_d[0, dk0 * P : (dk0 + DG) * P, :].rearrange(
                    "(k p) u -> p k u", p=P
                ),
            )
        for sb in range(1, NSB):
            nc.sync.dma_start(
                xT[:, :, sb * SB : (sb + 1) * SB],
                xt_d[sb].rearrange("(k p) u -> p k u", p=P),
            )

        # ---------------- rope position DMAs (gpsimd SWDGE, prefetched) ---
        posi = []
        for c in range(NCS):
            pi = const.tile([1, CS], i32, tag=f"rr_pi{c}", name=f"posi{c}")
            nc.gpsimd.dma_start(
                pi[:],
                pos_d.rearrange("(a s) -> a s", a=1)[:, c * CS : (c + 1) * CS],
            )
            posi.append(pi)

        # ---------------- constants ----------------
        ident = const.tile([P, P], bf16)
        make_identity(nc, ident)

        exp_bias = const.tile([P, 1], f32)
        nc.gpsimd.memset(exp_bias[:], EXP_BIAS)

        # causal additive mask for the diagonal [P, P] sub-block of a
        # scoresT tile: keep (0) where y >= x, else NEG.
        mask = const.tile([P, P], f32)
        nc.gpsimd.memset(mask[:], 0.0)
        nc.gpsimd.affine_select(
            out=mask[:],
            in_=mask[:],
            compare_op=mybir.AluOpType.is_ge,
            fill=NEG,
            base=0,
            pattern=[[1, P]],
            channel_multiplier=-1,
        )

        # ---------------- rope tables ----------------
        # inv_ts[i] = theta ** (-2 i / H), i in [0, HH)
        iot = const.tile([HH, 1], i32)
        nc.gpsimd.iota(iot[:], pattern=[[0, 1]], base=0, channel_multiplier=1)
        iotf = const.tile([HH, 1], f32)
        nc.vector.tensor_copy(iotf[:], iot[:])
        inv_ts = const.tile([HH, 1], f32)
        nc.scalar.activation(
            inv_ts[:], iotf[:], EXP, scale=-2.0 * math.log(theta) / H
        )

        TWO_PI = float(np.float32(2.0 * math.pi))
        PI = float(np.float32(math.pi))

        # cos2[h] = cos(angle_{h mod HH}); sin2s[h<HH] = -sin, sin2s[h>=HH] = +sin
        cos2 = persist.tile([P, S], f32)
        sin2s = persist.tile([P, S], f32)

        for c0 in range(0, S, CS):
            sl = slice(c0, c0 + CS)
            posf = rope_p.tile([1, CS], f32, tag="rr_pf", name="posf")
            nc.vector.tensor_copy(posf[:], posi[c0 // CS][:])
            pb = rope_p.tile([HH, CS], f32, tag="rr_pb", name="pb")
            nc.gpsimd.partition_broadcast(pb[:], posf[:])
            ang = rope_p.tile([HH, CS], f32, tag="rr_ang", name="ang")
            nc.vector.tensor_scalar_mul(ang[:], pb[:], inv_ts[:])

            def sin_reduced(dst, phase):
                # dst = sin(ang + phase).  k = int-cast((ang+phase)/2pi):
                # trunc (sim) gives red in [0, 2pi); round (hw) gives
                # [-pi, pi].  One conditional -2pi brings both to [-pi, pi].
                if phase != 0.0:
                    a = rope_p.tile([HH, CS], f32, tag="rr_a", name="a")
                    nc.vector.tensor_scalar_add(a[:], ang[:], phase)
                else:
                    a = ang
                t = rope_p.tile([HH, CS], f32, tag="rr_t", name="t")
                nc.vector.tensor_scalar_mul(t[:], a[:], 1.0 / TWO_PI)
                ki = rope_p.tile([HH, CS], i32, tag="rr_ki", name="ki")
                nc.vector.tensor_copy(ki[:], t[:])
                kf = rope_p.tile([HH, CS], f32, tag="rr_kf", name="kf")
                nc.vector.tensor_copy(kf[:], ki[:])
                red = rope_p.tile([HH, CS], f32, tag="rr_red", name="red")
                nc.vector.scalar_tensor_tensor(
                    red[:], kf[:], -TWO_PI, a[:], MULT, ADD
                )
                cc = rope_p.tile([HH, CS], f32, tag="rr_c", name="cc")
                nc.vector.tensor_scalar(
                    cc[:], red[:], PI, None, op0=mybir.AluOpType.is_gt
                )
                nc.vector.scalar_tensor_tensor(
                    red[:], cc[:], -TWO_PI, red[:], MULT, ADD
                )
                nc.scalar.activation(dst[:], red[:], SIN)

            sin_reduced(cos2[0:HH, sl], float(np.float32(math.pi / 2.0)))
            sin_reduced(sin2s[HH:P, sl], 0.0)  # +sin in hi half
            nc.vector.tensor_copy(cos2[HH:P, sl], cos2[0:HH, sl])
            nc.vector.tensor_scalar_mul(sin2s[0:HH, sl], sin2s[HH:P, sl], -1.0)

        # ---------------- projections for one block ----------------
        qT = [
            persist.tile([P, S], bf16, name=f"qT{h}", tag=f"qT{h}")
            for h in range(QH)
        ]
        kT = persist.tile([P, S], bf16)
        VW = H + 4
        vp = persist.tile([P, NT, VW], bf16)

        def proj_qk(w_sb, out_tile, sb):
            sl = slice(sb * SB, (sb + 1) * SB)
            pq = ps_sc.tile([P, SB], f32, tag="sc", name="pq")
            for dk in range(DK):
                nc.tensor.matmul(
                    pq[:],
                    w_sb[:, dk, :],
                    xT[:, dk, sl],
                    start=(dk == 0),
                    stop=(dk == DK - 1),
                )
            # rope: out = pq * cos2 + rot(pq) * sin2s
            tsin = small.tile([P, SB], f32, tag="tsin")
            nc.vector.tensor_tensor(
                tsin[0:HH, :], pq[HH:P, :], sin2s[0:HH, sl], MULT
            )
            nc.vector.tensor_tensor(
                tsin[HH:P, :], pq[0:HH, :], sin2s[HH:P, sl], MULT
            )
            tcos = small.tile([P, SB], f32, tag="tcos")
            nc.vector.tensor_tensor(tcos[:], pq[:], cos2[:, sl], MULT)
            nc.vector.tensor_tensor(out_tile[:, sl], tcos[:], tsin[:], ADD)

        def proj_block(sb):
            for h in range(QH):
                proj_qk(wq_sb[h], qT[h], sb)
            proj_qk(wk_sb, kT, sb)
            # v projection (v' with ones column), one [P, P] tile per tt
            for r in range(RB):
                tt = sb * RB + r
                pv = ps_sc.tile([P, P], f32, tag="sc", name="pv")
                for dk in range(DK):
                    nc.tensor.matmul(
                        pv[:],
                        xT[:, dk, tt * P : (tt + 1) * P],
                        wv_sb[:, dk, :],
                        start=(dk == 0),
                        stop=(dk == DK - 1),
                    )
                nc.vector.tensor_copy(vp[:, tt, 0:H], pv[:])
                nc.gpsimd.memset(vp[:, tt, H : H + 1], 1.0)

        # ---------------- attention + fused O projection ----------------
        # O-projection of block sb-1 is interleaved between the attention
        # heads of block sb so its PSUM-evict waits don't stall the PE queue.
        def oproj_tile(sb, attnT_blk, r2, orow):
            st = RB * sb + r2
            for db in range(D // SB):
                po = ps_sc.tile([P, SB], f32, tag="sc", name="po")
                for h in range(QH):
                    nc.tensor.matmul(
                        po[:],
                        attnT_blk[h][:, r2 * P : (r2 + 1) * P],
                        wo_sb[h][:, db * SB : (db + 1) * SB],
                        start=(h == 0),
                        stop=(h == QH - 1),
                    )
                nc.vector.tensor_copy(orow[:, db * SB : (db + 1) * SB], po[:])
            nc.sync.dma_start(o_d[st * P : (st + 1) * P, :], orow[:])

        def attention_head(sb, h, attnT):
            pav = [
                ps_av.tile(
                    [P, H + 1], f32, name=f"pav{r}", tag=f"av{r}", bufs=1
                )[:]
                for r in range(RB)
            ]
            ptr2 = ps_sc.tile([P, SB], bf16, tag="sc", name="ptr2")
            ans = [None] * RB

            def finish_subtile(r2):
                rec = small.tile([P, 1], f32, tag="rec", bufs=4)
                nc.vector.reciprocal(rec[:], pav[r2][:, H : H + 1])
                an = small.tile([P, H], bf16, tag="an", bufs=4)
                nc.vector.tensor_scalar_mul(an[:], pav[r2][:, 0:H], rec[:])
                ans[r2] = an

            def emit_transpose(r2):
                nc.tensor.transpose(
                    ptr2[:, r2 * P : (r2 + 1) * P], ans[r2][:], ident[:]
                )
                sl2 = slice(r2 * P, (r2 + 1) * P)
                nc.scalar.copy(attnT[h][:, sl2], ptr2[:, sl2])

            ntt = RB * (sb + 1)
            LOOK = 2  # scores lookahead: exp gets 2 tiles of lead time so
            # AV(tt) never stalls the in-order PE queue waiting on exp(tt)
            pscores = {}

            def emit_scores(tt):
                r = tt - RB * sb
                c0 = max(0, r) * P  # exact-causal: skip below-diag subtiles
                pscore = ps_sc.tile([P, SB], f32, tag="sc", name="pscore")
                nc.tensor.matmul(
                    pscore[:, c0:SB],
                    kT[:, tt * P : (tt + 1) * P],
                    qT[h][:, sb * SB + c0 : (sb + 1) * SB],
                    start=True,
                    stop=True,
                )
                if r >= 0:
                    nc.vector.tensor_tensor(
                        pscore[:, r * P : (r + 1) * P],
                        pscore[:, r * P : (r + 1) * P],
                        mask[:],
                        ADD,
                    )
                pscores[tt] = pscore

            for tt in range(min(LOOK, ntt)):
                emit_scores(tt)
            for tt in range(ntt):
                if tt + LOOK < ntt:
                    emit_scores(tt + LOOK)
                pscore = pscores.pop(tt)
                r = tt - RB * sb
                pt = pt_pool.tile([P, SB], bf16, tag="pt")
                c0 = max(0, r) * P
                nc.scalar.activation(
                    pt[:, c0:SB], pscore[:, c0:SB], EXP, bias=exp_bias[:]
                )
                for r2 in range(max(0, r), RB):
                    q128 = RB * sb + r2
                    nc.tensor.matmul(
                        pav[r2],
                        pt[:, r2 * P : (r2 + 1) * P],
                        vp[:, tt, 0 : H + 1],
                        start=(tt == 0),
                        stop=(tt == q128),
                    )
                if r >= 0:
                    finish_subtile(r)
                if r >= 1:
                    emit_transpose(r - 1)
            emit_transpose(RB - 1)

        prev = None
        for sb in range(NSB):
            proj_block(sb)
            attnT = [
                at_pool.tile(
                    [P, SB], bf16, name=f"attnT{h}", tag=f"attnT{h}", bufs=2
                )
                for h in range(QH)
            ]
            for h in range(QH):
                attention_head(sb, h, attnT)
                if prev is not None:
                    orow = ob_pool.tile([P, D], bf16, tag="ob")
                    oproj_tile(sb - 1, prev, h, orow)
            prev = attnT
        for r2 in range(RB):
            orow = ob_pool.tile([P, D], bf16, tag="ob")
            oproj_tile(NSB - 1, prev, r2, orow)

    nc.compile()
    return nc


_NC_CACHE = {}


def _get_nc(key):
    if key not in _NC_CACHE:
        _NC_CACHE[key] = build_nc(*key)
    return _NC_CACHE[key]


def make_in_maps(x, positions, Wq, Wk, Wv, Wo, n_cores=8):
    B, S, D = x.shape
    Q, _, H = Wq.shape
    N = Wk.shape[0]
    groups = Q // N if N else 1
    gpb = n_cores // B  # head groups per batch (4)
    qh_per_core = Q // gpb
    assert qh_per_core * gpb == Q
    scale = np.float32(1.0 / math.sqrt(H))
    SB = min(512, S)
    NSB = S // SB
    DK = D // P

    def shuf_w(w):  # [D, H] f32 -> [P, DK, H] bf16 (d = dk*P + p)
        return np.ascontiguousarray(
            w.reshape(DK, P, H).transpose(1, 0, 2).astype(ml_dtypes.bfloat16)
        )

    in_maps = []
    for c in range(n_cores):
        b = c // gpb
        g = c % gpb
        qh0 = g * qh_per_core
        kvh = qh0 // groups
        xt = np.ascontiguousarray(
            x[b].T.astype(ml_dtypes.bfloat16).reshape(D, NSB, SB).transpose(1, 0, 2)
        )
        in_maps.append(
            {
                "xt": xt,
                "positions": positions,
                "wq": np.stack(
                    [
                        shuf_w(Wq[qh0 + i] * scale)
                        for i in range(qh_per_core)
                    ]
                ),
                "wk": shuf_w(Wk[kvh]),
                "wv": shuf_w(Wv[kvh]),
                "wo": np.ascontiguousarray(
                    Wo[qh0 : qh0 + qh_per_core].astype(ml_dtypes.bfloat16)
                ),
            }
        )
    return in_maps, gpb, qh_per_core


def kernel(x, positions, Wq, Wk, Wv, Wo):
    """Full inputs -> full output.  x [B,S,D] f32, positions [S] i32,
    Wq [Q,D,H], Wk/Wv [N,D,H], Wo [Q,H,D].  Returns [B,S,D] f32."""
    from concourse.bass_utils import run_bass_kernel_spmd

    x = np.ascontiguousarray(np.asarray(x, dtype=np.float32))
    positions = np.ascontiguousarray(np.asarray(positions, dtype=np.int32))
    Wq = np.asarray(Wq, dtype=np.float32)
    Wk = np.asarray(Wk, dtype=np.float32)
    Wv = np.asarray(Wv, dtype=np.float32)
    Wo = np.asarray(Wo, dtype=np.float32)

    B, S, D = x.shape
    Q, _, H = Wq.shape
    n_cores = 8
    in_maps, gpb, qh_per_core = make_in_maps(x, positions, Wq, Wk, Wv, Wo, n_cores)

    nc = _get_nc((S, D, qh_per_core, H))
    res = run_bass_kernel_spmd(nc, in_maps, core_ids=list(range(n_cores)))
    out = np.zeros((B, S, D), dtype=np.float32)
    for c in range(n_cores):
        out[c // gpb] += res.results[c]["o"].astype(np.float32)
    return out


# revision 9
# speedup vs baseline: 1.2156x; 1.2156x over previous
"""Trainium2 Bass kernel for GQA attention (B=2, S=2048, D=2048, 16 q-heads,
4 kv-heads, head_dim=128, RoPE, causal) sharded over 8 NeuronCores.

Sharding: core c handles batch b = c//4 and q-head group g = c%4
(q-heads 4g..4g+3, which share kv-head g).  Each core computes a partial
output o_part[b] = sum_{its heads} attn_head @ Wo_head; the host sums the
4 partials per batch.

Schedule: fully software-pipelined over 512-row blocks.  Per block sb:
q/k/v projections (rope fused) -> attention heads 0..3 -> O-projection of
block sb-1 interleaved between heads.  All input DMAs ride the sync HWDGE
ring in exact consumption order (the ring serializes trigger-with-transfer,
so order IS the arrival schedule); rope tables are computed on the
otherwise-idle GpSimd engine so the DVE only does projection/attention
evictions.
"""

import sys

sys.path.insert(0, "/opt/trn_rl_repo")

import math

import ml_dtypes
import numpy as np

P = 128
NEG = -1.0e9
EXP_BIAS = -8.0  # exp(s - 8): cancels in softmax normalization, avoids overflow


def build_nc(S=2048, D=2048, QH=4, H=128, theta=10000.0):
    """Build the per-core Bass graph.

    Per-core problem: xt [NSB, D, SB] bf16 (x.T in 512-col blocks),
    positions [S] i32, wq [QH, P, DK, H] bf16 (pre-scaled by 1/sqrt(H),
    d-chunk-shuffled), wk/wv [P, DK, H] bf16, wo [QH, H, D] bf16
    ->  o [S, D] bf16 (partial over heads).
    """
    import concourse.bacc as bacc
    import concourse.mybir as mybir
    from concourse import tile
    from concourse.masks import make_identity

    f32 = mybir.dt.float32
    bf16 = mybir.dt.bfloat16
    i32 = mybir.dt.int32
    ADD = mybir.AluOpType.add
    MULT = mybir.AluOpType.mult
    EXP = mybir.ActivationFunctionType.Exp
    SIN = mybir.ActivationFunctionType.Sin

    assert H == P
    HH = H // 2  # 64
    DK = D // P  # d-chunks (16)
    SB = min(512, S)  # block width
    NSB = S // SB  # blocks (4)
    RB = SB // P  # 128-subtiles per block (4)
    NT = S // P  # t tiles (16)
    CS = min(256, S)  # rope table chunk width
    NCS = S // CS

    nc = bacc.Bacc(None, target_bir_lowering=False)

    xt_d = nc.declare_dram_parameter("xt", [NSB, D, SB], bf16, isOutput=False)
    pos_d = nc.declare_dram_parameter("positions", [S], i32, isOutput=False)
    wq_d = nc.declare_dram_parameter("wq", [QH, P, DK, H], bf16, isOutput=False)
    wk_d = nc.declare_dram_parameter("wk", [P, DK, H], bf16, isOutput=False)
    wv_d = nc.declare_dram_parameter("wv", [P, DK, H], bf16, isOutput=False)
    wo_d = nc.declare_dram_parameter("wo", [QH, H, D], bf16, isOutput=False)
    o_d = nc.declare_dram_parameter("o", [S, D], bf16, isOutput=True)

    from contextlib import ExitStack

    with tile.TileContext(nc) as tc, ExitStack() as es:
        # ---------------- pools ----------------
        const = es.enter_context(tc.tile_pool(name="const", bufs=1))
        persist = es.enter_context(tc.tile_pool(name="persist", bufs=1))
        small = es.enter_context(tc.tile_pool(name="small", bufs=2))
        rope_p = es.enter_context(tc.tile_pool(name="rope", bufs=1))
        pt_pool = es.enter_context(tc.tile_pool(name="pt", bufs=8))
        ob_pool = es.enter_context(tc.tile_pool(name="ob", bufs=2))
        at_pool = es.enter_context(tc.tile_pool(name="at", bufs=1))
        ps_sc = es.enter_context(tc.tile_pool(name="ps_sc", bufs=4, space="PSUM"))
        ps_av = es.enter_context(tc.tile_pool(name="ps_av", bufs=1, space="PSUM"))

        # ------- input DMAs: ALL on the sync ring, in consumption order ----
        # The HWDGE ring serializes trigger-with-transfer, so this order is
        # the arrival schedule: wq0 -> x block 0 (4 pieces for an early
        # start) -> wq1..3, wk, wv -> x block 1 -> wo -> x blocks 2, 3.
        wq_sb = [
            persist.tile([P, DK, H], bf16, name=f"wq{h}", tag=f"wq{h}")
            for h in range(QH)
        ]
        wk_sb = persist.tile([P, DK, H], bf16, name="wk", tag="wk")
        wv_sb = persist.tile([P, DK, H], bf16, name="wv", tag="wv")
        wo_sb = [
            persist.tile([P, D], bf16, name=f"wo{h}", tag=f"wo{h}")
            for h in range(QH)
        ]
        xT = persist.tile([P, DK, S], bf16)

        def dma_x_block(sb, ngroups=1):
            DG = DK // ngroups
            for dk0 in range(0, DK, DG):
                nc.sync.dma_start(
                    xT[:, dk0 : dk0 + DG, sb * SB : (sb + 1) * SB],
                    xt_d[sb, dk0 * P : (dk0 + DG) * P, :].rearrange(
                        "(k p) u -> p k u", p=P
                    ),
                )

        nc.sync.dma_start(wq_sb[0][:], wq_d[0])
        dma_x_block(0, ngroups=4)
        for h in range(1, QH):
            nc.sync.dma_start(wq_sb[h][:], wq_d[h])
        nc.sync.dma_start(wk_sb[:], wk_d[:])
        nc.sync.dma_start(wv_sb[:], wv_d[:])
        dma_x_block(1)
        for h in range(QH):
            nc.sync.dma_start(wo_sb[h][:], wo_d[h])
        dma_x_block(2)
        dma_x_block(3)

        # ---------------- rope position DMAs (gpsimd SWDGE, prefetched) ---
        posi = []
        for c in range(NCS):
            pi = const.tile([1, CS], i32, tag=f"rr_pi{c}", name=f"posi{c}")
            nc.gpsimd.dma_start(
                pi[:],
                pos_d.rearrange("(a s) -> a s", a=1)[:, c * CS : (c + 1) * CS],
            )
            posi.append(pi)

        # ---------------- constants (gpsimd) ----------------
        ident = const.tile([P, P], bf16)
        make_identity(nc, ident)

        exp_bias = const.tile([P, 1], f32)
        nc.gpsimd.memset(exp_bias[:], EXP_BIAS)

        # causal additive mask for the diagonal [P, P] sub-block of a
        # scoresT tile: keep (0) where y >= x, else NEG.
        mask = const.tile([P, P], f32)
        nc.gpsimd.memset(mask[:], 0.0)
        nc.gpsimd.affine_select(
            out=mask[:],
            in_=mask[:],
            compare_op=mybir.AluOpType.is_ge,
            fill=NEG,
            base=0,
            pattern=[[1, P]],
            channel_multiplier=-1,
        )

        # ---------------- rope tables (DVE + Scalar sin) ----------------
        # inv_ts[i] = theta ** (-2 i / H), i in [0, HH)
        iot = const.tile([HH, 1], i32)
        nc.gpsimd.iota(iot[:], pattern=[[0, 1]], base=0, channel_multiplier=1)
        iotf = const.tile([HH, 1], f32)
        nc.gpsimd.tensor_copy(iotf[:], iot[:])
        inv_ts = const.tile([HH, 1], f32)
        nc.scalar.activation(
            inv_ts[:], iotf[:], EXP, scale=-2.0 * math.log(theta) / H
        )

        TWO_PI = float(np.float32(2.0 * math.pi))
        PI = float(np.float32(math.pi))

        # cos2[h] = cos(angle_{h mod HH}); sin2s[h<HH] = -sin, sin2s[h>=HH] = +sin
        cos2 = persist.tile([P, S], f32)
        sin2s = persist.tile([P, S], f32)

        def rope_chunk(c):
            """Emit the table chain for s in [c*CS, (c+1)*CS).  Called at
            block starts so the ~2.5us DVE chain hides under the block's
            projection matmuls instead of clogging attention evictions."""
            c0 = c * CS
            sl = slice(c0, c0 + CS)
            posf = rope_p.tile([1, CS], f32, tag="rr_pf", name="posf")
            nc.vector.tensor_copy(posf[:], posi[c][:])
            pb = rope_p.tile([HH, CS], f32, tag="rr_pb", name="pb")
            nc.gpsimd.partition_broadcast(pb[:], posf[:])
            ang = rope_p.tile([HH, CS], f32, tag="rr_ang", name="ang")
            nc.vector.tensor_scalar_mul(ang[:], pb[:], inv_ts[:])

            def sin_reduced(dst, phase):
                # dst = sin(ang + phase).  k = int-cast((ang+phase)/2pi):
                # trunc (sim) gives red in [0, 2pi); round (hw) gives
                # [-pi, pi].  One conditional -2pi brings both to [-pi, pi].
                if phase != 0.0:
                    a = rope_p.tile([HH, CS], f32, tag="rr_a", name="a")
                    nc.vector.tensor_scalar_add(a[:], ang[:], phase)
                else:
                    a = ang
                t = rope_p.tile([HH, CS], f32, tag="rr_t", name="t")
                nc.vector.tensor_scalar_mul(t[:], a[:], 1.0 / TWO_PI)
                ki = rope_p.tile([HH, CS], i32, tag="rr_ki", name="ki")
                nc.vector.tensor_copy(ki[:], t[:])
                kf = rope_p.tile([HH, CS], f32, tag="rr_kf", name="kf")
                nc.vector.tensor_copy(kf[:], ki[:])
                red = rope_p.tile([HH, CS], f32, tag="rr_red", name="red")
                nc.vector.scalar_tensor_tensor(
                    red[:], kf[:], -TWO_PI, a[:], MULT, ADD
                )
                cc = rope_p.tile([HH, CS], f32, tag="rr_c", name="cc")
                nc.vector.tensor_scalar(
                    cc[:], red[:], PI, None, op0=mybir.AluOpType.is_gt
                )
                nc.vector.scalar_tensor_tensor(
                    red[:], cc[:], -TWO_PI, red[:], MULT, ADD
                )
                nc.scalar.activation(dst[:], red[:], SIN)

            sin_reduced(cos2[0:HH, sl], float(np.float32(math.pi / 2.0)))
            sin_reduced(sin2s[HH:P, sl], 0.0)  # +sin in hi half
            nc.vector.tensor_copy(cos2[HH:P, sl], cos2[0:HH, sl])
            nc.vector.tensor_scalar_mul(sin2s[0:HH, sl], sin2s[HH:P, sl], -1.0)

        # ---------------- projections for one block ----------------
        qT = [
            persist.tile([P, S], bf16, name=f"qT{h}", tag=f"qT{h}")
            for h in range(QH)
        ]
        kT = persist.tile([P, S], bf16)
        VW = H + 4
        vp = persist.tile([P, NT, VW], bf16)

        def proj_qk(w_sb, out_tile, sb):
            sl = slice(sb * SB, (sb + 1) * SB)
            pq = ps_sc.tile([P, SB], f32, tag="sc", name="pq")
            for dk in range(DK):
                nc.tensor.matmul(
                    pq[:],
                    w_sb[:, dk, :],
                    xT[:, dk, sl],
                    start=(dk == 0),
                    stop=(dk == DK - 1),
                )
            # rope: out = pq * cos2 + rot(pq) * sin2s
            tsin = small.tile([P, SB], f32, tag="tsin")
            nc.vector.tensor_tensor(
                tsin[0:HH, :], pq[HH:P, :], sin2s[0:HH, sl], MULT
            )
            nc.vector.tensor_tensor(
                tsin[HH:P, :], pq[0:HH, :], sin2s[HH:P, sl], MULT
            )
            tcos = small.tile([P, SB], f32, tag="tcos")
            nc.vector.tensor_tensor(tcos[:], pq[:], cos2[:, sl], MULT)
            nc.vector.tensor_tensor(out_tile[:, sl], tcos[:], tsin[:], ADD)

        def proj_block(sb):
            # this block's rope-table chunks first: their DVE chain runs
            # while the first projection matmuls fill PSUM
            for c in range(sb * SB // CS, (sb + 1) * SB // CS):
                rope_chunk(c)
            for h in range(QH):
                proj_qk(wq_sb[h], qT[h], sb)
            proj_qk(wk_sb, kT, sb)
            # v projection (v' with ones column), one [P, P] tile per tt
            for r in range(RB):
                tt = sb * RB + r
                pv = ps_sc.tile([P, P], f32, tag="sc", name="pv")
                for dk in range(DK):
                    nc.tensor.matmul(
                        pv[:],
                        xT[:, dk, tt * P : (tt + 1) * P],
                        wv_sb[:, dk, :],
                        start=(dk == 0),
                        stop=(dk == DK - 1),
                    )
                nc.vector.tensor_copy(vp[:, tt, 0:H], pv[:])
                nc.gpsimd.memset(vp[:, tt, H : H + 1], 1.0)

        # ---------------- attention + fused O projection ----------------
        # O-projection of block sb-1 is interleaved between the attention
        # heads of block sb so its PSUM-evict waits don't stall the PE queue.
        def oproj_tile(sb, attnT_blk, r2, orow):
            st = RB * sb + r2
            for db in range(D // SB):
                po = ps_sc.tile([P, SB], f32, tag="sc", name="po")
                for h in range(QH):
                    nc.tensor.matmul(
                        po[:],
                        attnT_blk[h][:, r2 * P : (r2 + 1) * P],
                        wo_sb[h][:, db * SB : (db + 1) * SB],
                        start=(h == 0),
                        stop=(h == QH - 1),
                    )
                nc.vector.tensor_copy(orow[:, db * SB : (db + 1) * SB], po[:])
            nc.sync.dma_start(o_d[st * P : (st + 1) * P, :], orow[:])

        def attention_head(sb, h, attnT):
            pav = [
                ps_av.tile(
                    [P, H + 1], f32, name=f"pav{r}", tag=f"av{r}", bufs=1
                )[:]
                for r in range(RB)
            ]
            ptr2 = ps_sc.tile([P, SB], bf16, tag="sc", name="ptr2")
            ans = [None] * RB

            def finish_subtile(r2):
                rec = small.tile([P, 1], f32, tag="rec", bufs=4)
                nc.vector.reciprocal(rec[:], pav[r2][:, H : H + 1])
                an = small.tile([P, H], bf16, tag="an", bufs=4)
                nc.vector.tensor_scalar_mul(an[:], pav[r2][:, 0:H], rec[:])
                ans[r2] = an

            def emit_transpose(r2):
                nc.tensor.transpose(
                    ptr2[:, r2 * P : (r2 + 1) * P], ans[r2][:], ident[:]
                )
                sl2 = slice(r2 * P, (r2 + 1) * P)
                nc.scalar.copy(attnT[h][:, sl2], ptr2[:, sl2])

            ntt = RB * (sb + 1)
            LOOK = 2  # scores lookahead: exp gets 2 tiles of lead time so
            # AV(tt) never stalls the in-order PE queue waiting on exp(tt)
            pscores = {}

            def emit_scores(tt):
                r = tt - RB * sb
                c0 = max(0, r) * P  # exact-causal: skip below-diag subtiles
                pscore = ps_sc.tile([P, SB], f32, tag="sc", name="pscore")
                nc.tensor.matmul(
                    pscore[:, c0:SB],
                    kT[:, tt * P : (tt + 1) * P],
                    qT[h][:, sb * SB + c0 : (sb + 1) * SB],
                    start=True,
                    stop=True,
                )
                if r >= 0:
                    nc.vector.tensor_tensor(
                        pscore[:, r * P : (r + 1) * P],
                        pscore[:, r * P : (r + 1) * P],
                        mask[:],
                        ADD,
                    )
                pscores[tt] = pscore

            for tt in range(min(LOOK, ntt)):
                emit_scores(tt)
            for tt in range(ntt):
                if tt + LOOK < ntt:
                    emit_scores(tt + LOOK)
                pscore = pscores.pop(tt)
                r = tt - RB * sb
                pt = pt_pool.tile([P, SB], bf16, tag="pt")
                c0 = max(0, r) * P
                nc.scalar.activation(
                    pt[:, c0:SB], pscore[:, c0:SB], EXP, bias=exp_bias[:]
                )
                for r2 in range(max(0, r), RB):
                    q128 = RB * sb + r2
                    nc.tensor.matmul(
                        pav[r2],
                        pt[:, r2 * P : (r2 + 1) * P],
                        vp[:, tt, 0 : H + 1],
                        start=(tt == 0),
                        stop=(tt == q128),
                    )
                if r >= 0:
                    finish_subtile(r)
                if r >= 1:
                    emit_transpose(r - 1)
            emit_transpose(RB - 1)

        prev = None
        for sb in range(NSB):
            proj_block(sb)
            attnT = [
                at_pool.tile(
                    [P, SB], bf16, name=f"attnT{h}", tag=f"attnT{h}", bufs=2
                )
                for h in range(QH)
            ]
            for h in range(QH):
                attention_head(sb, h, attnT)
                if prev is not None:
                    orow = ob_pool.tile([P, D], bf16, tag="ob")
                    oproj_tile(sb - 1, prev, h, orow)
            prev = attnT
        for r2 in range(RB):
            orow = ob_pool.tile([P, D], bf16, tag="ob")
            oproj_tile(NSB - 1, prev, r2, orow)

    nc.compile()
    return nc


_NC_CACHE = {}


def _get_nc(key):
    if key not in _NC_CACHE:
        _NC_CACHE[key] = build_nc(*key)
    return _NC_CACHE[key]


def make_in_maps(x, positions, Wq, Wk, Wv, Wo, n_cores=8):
    B, S, D = x.shape
    Q, _, H = Wq.shape
    N = Wk.shape[0]
    groups = Q // N if N else 1
    gpb = n_cores // B  # head groups per batch (4)
    qh_per_core = Q // gpb
    assert qh_per_core * gpb == Q
    scale = np.float32(1.0 / math.sqrt(H))
    SB = min(512, S)
    NSB = S // SB
    DK = D // P

    def shuf_w(w):  # [D, H] f32 -> [P, DK, H] bf16 (d = dk*P + p)
        return np.ascontiguousarray(
            w.reshape(DK, P, H).transpose(1, 0, 2).astype(ml_dtypes.bfloat16)
        )

    in_maps = []
    for c in range(n_cores):
        b = c // gpb
        g = c % gpb
        qh0 = g * qh_per_core
        kvh = qh0 // groups
        xt = np.ascontiguousarray(
            x[b].T.astype(ml_dtypes.bfloat16).reshape(D, NSB, SB).transpose(1, 0, 2)
        )
        in_maps.append(
            {
                "xt": xt,
                "positions": positions,
                "wq": np.stack(
                    [
                        shuf_w(Wq[qh0 + i] * scale)
                        for i in range(qh_per_core)
                    ]
                ),
                "wk": shuf_w(Wk[kvh]),
                "wv": shuf_w(Wv[kvh]),
                "wo": np.ascontiguousarray(
                    Wo[qh0 : qh0 + qh_per_core].astype(ml_dtypes.bfloat16)
                ),
            }
        )
    return in_maps, gpb, qh_per_core


def kernel(x, positions, Wq, Wk, Wv, Wo):
    """Full inputs -> full output.  x [B,S,D] f32, positions [S] i32,
    Wq [Q,D,H], Wk/Wv [N,D,H], Wo [Q,H,D].  Returns [B,S,D] f32."""
    from concourse.bass_utils import run_bass_kernel_spmd

    x = np.ascontiguousarray(np.asarray(x, dtype=np.float32))
    positions = np.ascontiguousarray(np.asarray(positions, dtype=np.int32))
    Wq = np.asarray(Wq, dtype=np.float32)
    Wk = np.asarray(Wk, dtype=np.float32)
    Wv = np.asarray(Wv, dtype=np.float32)
    Wo = np.asarray(Wo, dtype=np.float32)

    B, S, D = x.shape
    Q, _, H = Wq.shape
    n_cores = 8
    in_maps, gpb, qh_per_core = make_in_maps(x, positions, Wq, Wk, Wv, Wo, n_cores)

    nc = _get_nc((S, D, qh_per_core, H))
    res = run_bass_kernel_spmd(nc, in_maps, core_ids=list(range(n_cores)))
    out = np.zeros((B, S, D), dtype=np.float32)
    for c in range(n_cores):
        out[c // gpb] += res.results[c]["o"].astype(np.float32)
    return out


# revision 25
# speedup vs baseline: 1.3551x; 1.1147x over previous
"""Trainium2 Bass kernel for GQA attention (B=2, S=2048, D=2048, 16 q-heads,
4 kv-heads, head_dim=128, RoPE, causal) sharded over 8 NeuronCores.

Sharding: core c handles batch b = c//4 and q-head group g = c%4
(q-heads 4g..4g+3, which share kv-head g).  Each core computes a partial
output o_part[b] = sum_{its heads} attn_head @ Wo_head; the host sums the
4 partials per batch.

Schedule: fully software-pipelined over 512-row blocks.  Per block sb:
q/k/v projections (rope fused) -> attention heads 0..3 -> O-projection of
block sb-1 interleaved between heads.  All input DMAs ride the sync HWDGE
ring in exact consumption order (the ring serializes trigger-with-transfer,
so order IS the arrival schedule); rope tables are computed on the
otherwise-idle GpSimd engine so the DVE only does projection/attention
evictions.
"""

import sys

sys.path.insert(0, "/opt/trn_rl_repo")

import math

import ml_dtypes
import numpy as np

P = 128
NEG = -1.0e9
EXP_BIAS = -8.0  # exp(s - 8): cancels in softmax normalization, avoids overflow


def build_nc(S=2048, D=2048, QH=4, H=128, theta=10000.0):
    """Build the per-core Bass graph.

    Per-core problem: xt [NSB, D, SB] bf16 (x.T in 512-col blocks),
    positions [S] i32, wq [QH, P, DK, H] bf16 (pre-scaled by 1/sqrt(H),
    d-chunk-shuffled), wk/wv [P, DK, H] bf16, wo [QH, H, D] bf16
    ->  o [S, D] bf16 (partial over heads).
    """
    import concourse.bacc as bacc
    import concourse.mybir as mybir
    from concourse import tile
    from concourse.masks import make_identity

    f32 = mybir.dt.float32
    bf16 = mybir.dt.bfloat16
    i32 = mybir.dt.int32
    ADD = mybir.AluOpType.add
    MULT = mybir.AluOpType.mult
    EXP = mybir.ActivationFunctionType.Exp
    SIN = mybir.ActivationFunctionType.Sin

    assert H == P
    HH = H // 2  # 64
    DK = D // P  # d-chunks (16)
    SB = min(512, S)  # block width
    NSB = S // SB  # blocks (4)
    RB = SB // P  # 128-subtiles per block (4)
    NT = S // P  # t tiles (16)
    CS = min(256, S)  # rope table chunk width
    NCS = S // CS

    nc = bacc.Bacc(None, target_bir_lowering=False)

    xt_d = nc.declare_dram_parameter("xt", [NSB, P, DK, SB], bf16, isOutput=False)
    cos_d = nc.declare_dram_parameter("cos2", [P, S], bf16, isOutput=False)
    sin_d = nc.declare_dram_parameter("sin2s", [P, S], bf16, isOutput=False)
    wq_d = nc.declare_dram_parameter("wq", [QH, P, DK, H], bf16, isOutput=False)
    wk_d = nc.declare_dram_parameter("wk", [P, DK, H], bf16, isOutput=False)
    wv_d = nc.declare_dram_parameter("wv", [P, DK, H], bf16, isOutput=False)
    wo_d = nc.declare_dram_parameter("wo", [QH, H, D], bf16, isOutput=False)
    o_d = nc.declare_dram_parameter("o", [S, D], bf16, isOutput=True)

    from contextlib import ExitStack

    with tile.TileContext(nc) as tc, ExitStack() as es:
        # ---------------- pools ----------------
        const = es.enter_context(tc.tile_pool(name="const", bufs=1))
        persist = es.enter_context(tc.tile_pool(name="persist", bufs=1))
        small = es.enter_context(tc.tile_pool(name="small", bufs=2))
        pt_pool = es.enter_context(tc.tile_pool(name="pt", bufs=8))
        ob_pool = es.enter_context(tc.tile_pool(name="ob", bufs=2))
        at_pool = es.enter_context(tc.tile_pool(name="at", bufs=1))
        ps_sc = es.enter_context(tc.tile_pool(name="ps_sc", bufs=4, space="PSUM"))
        ps_av = es.enter_context(tc.tile_pool(name="ps_av", bufs=1, space="PSUM"))

        # ------- input DMAs: ALL on the sync ring, in consumption order ----
        # The HWDGE ring serializes trigger-with-transfer, so this order is
        # the arrival schedule: wq0 -> x block 0 (4 pieces for an early
        # start) -> wq1..3, wk, wv -> x block 1 -> wo -> x blocks 2, 3.
        wq_sb = [
            persist.tile([P, DK, H], bf16, name=f"wq{h}", tag=f"wq{h}")
            for h in range(QH)
        ]
        wk_sb = persist.tile([P, DK, H], bf16, name="wk", tag="wk")
        wv_sb = persist.tile([P, DK, H], bf16, name="wv", tag="wv")
        wo_sb = [
            persist.tile([P, D], bf16, name=f"wo{h}", tag=f"wo{h}")
            for h in range(QH)
        ]
        # block-major free layout: each x block (and each dk-group piece of
        # block 0) is one CONTIGUOUS interval of the tile, so the tile
        # framework's interval-based dependency tracking never creates false
        # cross-block waits (the [P, DK, S] layout interleaved blocks with
        # dk stripes and serialized projections behind later block DMAs)
        xT = persist.tile([P, NSB, DK, SB], bf16)

        def dma_x_block(sb, ngroups=1):
            DG = DK // ngroups
            for dk0 in range(0, DK, DG):
                nc.sync.dma_start(
                    xT[:, sb, dk0 : dk0 + DG, :],
                    xt_d[sb, :, dk0 : dk0 + DG, :],
                )

        cos2 = persist.tile([P, S], bf16)
        sin2s = persist.tile([P, S], bf16)

        # x + projection weights on the sync ring in consumption order;
        # rope tables (bf16) + wo on the scalar ring.  One ring alone cannot
        # saturate HBM (~40us slower measured), so both run concurrently.
        nc.sync.dma_start(wq_sb[0][:], wq_d[0])
        dma_x_block(0, ngroups=4)
        for h in range(1, QH):
            nc.sync.dma_start(wq_sb[h][:], wq_d[h])
        nc.sync.dma_start(wk_sb[:], wk_d[:])
        nc.sync.dma_start(wv_sb[:], wv_d[:])
        dma_x_block(1)
        dma_x_block(2)
        dma_x_block(3)
        nc.scalar.dma_start(cos2[:], cos_d[:])
        nc.scalar.dma_start(sin2s[:], sin_d[:])
        for h in range(QH):
            nc.scalar.dma_start(wo_sb[h][:], wo_d[h])

        # ---------------- constants (gpsimd) ----------------
        ident = const.tile([P, P], bf16)
        make_identity(nc, ident)

        exp_bias = const.tile([P, 1], f32)
        nc.gpsimd.memset(exp_bias[:], EXP_BIAS)

        # causal additive mask for the diagonal [P, P] sub-block of a
        # scoresT tile: keep (0) where y >= x, else NEG.
        mask = const.tile([P, P], f32)
        nc.gpsimd.memset(mask[:], 0.0)
        nc.gpsimd.affine_select(
            out=mask[:],
            in_=mask[:],
            compare_op=mybir.AluOpType.is_ge,
            fill=NEG,
            base=0,
            pattern=[[1, P]],
            channel_multiplier=-1,
        )

        # ---------------- projections for one block ----------------
        qT = [
            persist.tile([P, S], bf16, name=f"qT{h}", tag=f"qT{h}")
            for h in range(QH)
        ]
        kT = persist.tile([P, S], bf16)
        VW = H + 4
        vp = persist.tile([P, NT, VW], bf16)

        def proj_qk(w_sb, out_tile, sb):
            sl = slice(sb * SB, (sb + 1) * SB)
            pq = ps_sc.tile([P, SB], f32, tag="sc", name="pq")
            for dk in range(DK):
                nc.tensor.matmul(
                    pq[:],
                    w_sb[:, dk, :],
                    xT[:, sb, dk, :],
                    start=(dk == 0),
                    stop=(dk == DK - 1),
                )
            # rope: out = pq * cos2 + rot(pq) * sin2s
            tsin = small.tile([P, SB], f32, tag="tsin")
            nc.vector.tensor_tensor(
                tsin[0:HH, :], pq[HH:P, :], sin2s[0:HH, sl], MULT
            )
            nc.vector.tensor_tensor(
                tsin[HH:P, :], pq[0:HH, :], sin2s[HH:P, sl], MULT
            )
            tcos = small.tile([P, SB], f32, tag="tcos")
            nc.vector.tensor_tensor(tcos[:], pq[:], cos2[:, sl], MULT)
            nc.vector.tensor_tensor(out_tile[:, sl], tcos[:], tsin[:], ADD)

        def proj_block(sb):
            for h in range(QH):
                proj_qk(wq_sb[h], qT[h], sb)
            proj_qk(wk_sb, kT, sb)
            # v projection (v' with ones column), one [P, P] tile per tt
            for r in range(RB):
                tt = sb * RB + r
                pv = ps_sc.tile([P, P], f32, tag="sc", name="pv")
                for dk in range(DK):
                    nc.tensor.matmul(
                        pv[:],
                        xT[:, sb, dk, r * P : (r + 1) * P],
                        wv_sb[:, dk, :],
                        start=(dk == 0),
                        stop=(dk == DK - 1),
                    )
                nc.vector.tensor_copy(vp[:, tt, 0:H], pv[:])
                nc.gpsimd.memset(vp[:, tt, H : H + 1], 1.0)

        # ---------------- attention + fused O projection ----------------
        # O-projection of block sb-1 is interleaved between the attention
        # heads of block sb so its PSUM-evict waits don't stall the PE queue.
        def oproj_tile(sb, attnT_blk, r2, orow):
            st = RB * sb + r2
            for db in range(D // SB):
                po = ps_sc.tile([P, SB], f32, tag="sc", name="po")
                for h in range(QH):
                    nc.tensor.matmul(
                        po[:],
                        attnT_blk[h][:, r2 * P : (r2 + 1) * P],
                        wo_sb[h][:, db * SB : (db + 1) * SB],
                        start=(h == 0),
                        stop=(h == QH - 1),
                    )
                if db < 2:
                    nc.scalar.copy(orow[:, db * SB : (db + 1) * SB], po[:])
                else:
                    nc.vector.tensor_copy(orow[:, db * SB : (db + 1) * SB], po[:])
            nc.sync.dma_start(o_d[st * P : (st + 1) * P, :], orow[:])

        def prefetch_scores(sb, h, look=2):
            """Emit head h's first `look` score tiles early (before the
            preceding O-projection tile) so exp has lead time over the
            head boundary.  Must mirror emit_scores exactly, including the
            diagonal-subtile mask."""
            pre = {}
            ntt = RB * (sb + 1)
            for tt in range(min(look, ntt)):
                r = tt - RB * sb
                c0 = max(0, r) * P
                pscore = ps_sc.tile([P, SB], f32, tag="sc", name="pscore")
                nc.tensor.matmul(
                    pscore[:, c0:SB],
                    kT[:, tt * P : (tt + 1) * P],
                    qT[h][:, sb * SB + c0 : (sb + 1) * SB],
                    start=True,
                    stop=True,
                )
                if r >= 0:
                    nc.vector.tensor_tensor(
                        pscore[:, r * P : (r + 1) * P],
                        pscore[:, r * P : (r + 1) * P],
                        mask[:],
                        ADD,
                    )
                pre[tt] = pscore
            return pre

        def attention_head(sb, h, attnT, post_transpose=None, pre=None):
            # one PSUM bank per AV accumulator: start=True clears has_written
            # for the WHOLE bank, so accumulation groups can never share one
            pav = [
                ps_av.tile(
                    [P, H + 1], f32, name=f"pav{r}", tag=f"av{r}", bufs=1
                )[:]
                for r in range(RB)
            ]
            ptr2 = ps_sc.tile([P, SB], bf16, tag="sc", name="ptr2")
            ans = [None] * RB

            def finish_subtile(r2):
                rec = small.tile([P, 1], f32, tag="rec", bufs=4)
                nc.vector.reciprocal(rec[:], pav[r2][:, H : H + 1])
                an = small.tile([P, H], bf16, tag="an", bufs=4)
                nc.vector.tensor_scalar_mul(an[:], pav[r2][:, 0:H], rec[:])
                ans[r2] = an

            def emit_transpose(r2):
                nc.tensor.transpose(
                    ptr2[:, r2 * P : (r2 + 1) * P], ans[r2][:], ident[:]
                )
                sl2 = slice(r2 * P, (r2 + 1) * P)
                nc.vector.tensor_copy(attnT[h][:, sl2], ptr2[:, sl2])
                if post_transpose is not None:
                    post_transpose(r2)

            ntt = RB * (sb + 1)
            LOOK = 2  # scores lookahead: exp gets 2 tiles of lead time so
            # AV(tt) never stalls the in-order PE queue waiting on exp(tt)
            pscores = pre if pre is not None else {}

            def emit_scores(tt):
                r = tt - RB * sb
                c0 = max(0, r) * P  # exact-causal: skip below-diag subtiles
                pscore = ps_sc.tile([P, SB], f32, tag="sc", name="pscore")
                nc.tensor.matmul(
                    pscore[:, c0:SB],
                    kT[:, tt * P : (tt + 1) * P],
                    qT[h][:, sb * SB + c0 : (sb + 1) * SB],
                    start=True,
                    stop=True,
                )
                if r >= 0:
                    nc.vector.tensor_tensor(
                        pscore[:, r * P : (r + 1) * P],
                        pscore[:, r * P : (r + 1) * P],
                        mask[:],
                        ADD,
                    )
                pscores[tt] = pscore

            for tt in range(min(LOOK, ntt)):
                if tt not in pscores:
                    emit_scores(tt)
            for tt in range(ntt):
                if tt + LOOK < ntt:
                    emit_scores(tt + LOOK)
                pscore = pscores.pop(tt)
                r = tt - RB * sb
                pt = pt_pool.tile([P, SB], bf16, tag="pt")
                c0 = max(0, r) * P
                nc.scalar.activation(
                    pt[:, c0:SB], pscore[:, c0:SB], EXP, bias=exp_bias[:]
                )
                for r2 in range(max(0, r), RB):
                    q128 = RB * sb + r2
                    nc.tensor.matmul(
                        pav[r2],
                        pt[:, r2 * P : (r2 + 1) * P],
                        vp[:, tt, 0 : H + 1],
                        start=(tt == 0),
                        stop=(tt == q128),
                    )
                if r >= 0:
                    finish_subtile(r)
                if r >= 1:
                    emit_transpose(r - 1)
            emit_transpose(RB - 1)

        prev = None
        for sb in range(NSB):
            proj_block(sb)
            attnT = [
                at_pool.tile(
                    [P, SB], bf16, name=f"attnT{h}", tag=f"attnT{h}", bufs=2
                )
                for h in range(QH)
            ]
            last = sb == NSB - 1
            pre = None
            for h in range(QH):
                if last and prev is not None:
                    orow = ob_pool.tile([P, D], bf16, tag="ob")
                    oproj_tile(sb - 1, prev, h, orow)
                post = None
                if last and h == QH - 1:
                    def post(r2, _attnT=attnT):
                        orow = ob_pool.tile([P, D], bf16, tag="ob")
                        oproj_tile(NSB - 1, _attnT, r2, orow)
                attention_head(sb, h, attnT, post_transpose=post, pre=pre)
                pre = prefetch_scores(sb, h + 1) if h + 1 < QH else None
                if not last and prev is not None:
                    orow = ob_pool.tile([P, D], bf16, tag="ob")
                    oproj_tile(sb - 1, prev, h, orow)
            prev = attnT

    nc.compile()
    return nc


_NC_CACHE = {}


def _get_nc(key):
    if key not in _NC_CACHE:
        _NC_CACHE[key] = build_nc(*key)
    return _NC_CACHE[key]


def make_in_maps(x, positions, Wq, Wk, Wv, Wo, n_cores=8):
    B, S, D = x.shape
    Q, _, H = Wq.shape
    N = Wk.shape[0]
    groups = Q // N if N else 1
    gpb = n_cores // B  # head groups per batch (4)
    qh_per_core = Q // gpb
    assert qh_per_core * gpb == Q
    scale = np.float32(1.0 / math.sqrt(H))
    SB = min(512, S)
    NSB = S // SB
    DK = D // P

    def shuf_w(w):  # [D, H] f32 -> [P, DK, H] bf16 (d = dk*P + p)
        return np.ascontiguousarray(
            w.reshape(DK, P, H).transpose(1, 0, 2).astype(ml_dtypes.bfloat16)
        )

    in_maps = []
    for c in range(n_cores):
        b = c // gpb
        g = c % gpb
        qh0 = g * qh_per_core
        kvh = qh0 // groups
        # [NSB, P, DK, SB]: 4KB+ contiguous segments on BOTH DMA sides
        xt = np.ascontiguousarray(
            x[b]
            .astype(ml_dtypes.bfloat16)
            .reshape(NSB, SB, DK, P)
            .transpose(0, 3, 2, 1)
        )
        in_maps.append(
            {
                "xt": xt,
                "positions": positions,
                "wq": np.stack(
                    [
                        shuf_w(Wq[qh0 + i] * scale)
                        for i in range(qh_per_core)
                    ]
                ),
                "wk": shuf_w(Wk[kvh]),
                "wv": shuf_w(Wv[kvh]),
                "wo": np.ascontiguousarray(
                    Wo[qh0 : qh0 + qh_per_core].astype(ml_dtypes.bfloat16)
                ),
            }
        )
    return in_maps, gpb, qh_per_core


def kernel(x, positions, Wq, Wk, Wv, Wo):
    """Full inputs -> full output.  x [B,S,D] f32, positions [S] i32,
    Wq [Q,D,H], Wk/Wv [N,D,H], Wo [Q,H,D].  Returns [B,S,D] f32."""
    from concourse.bass_utils import run_bass_kernel_spmd

    x = np.ascontiguousarray(np.asarray(x, dtype=np.float32))
    positions = np.ascontiguousarray(np.asarray(positions, dtype=np.int32))
    Wq = np.asarray(Wq, dtype=np.float32)
    Wk = np.asarray(Wk, dtype=np.float32)
    Wv = np.asarray(Wv, dtype=np.float32)
    Wo = np.asarray(Wo, dtype=np.float32)

    B, S, D = x.shape
    Q, _, H = Wq.shape
    n_cores = 8
    in_maps, gpb, qh_per_core = make_in_maps(x, positions, Wq, Wk, Wv, Wo, n_cores)

    nc = _get_nc((S, D, qh_per_core, H))
    res = run_bass_kernel_spmd(nc, in_maps, core_ids=list(range(n_cores)))
    out = np.zeros((B, S, D), dtype=np.float32)
    for c in range(n_cores):
        out[c // gpb] += res.results[c]["o"].astype(np.float32)
    return out


# revision 26
# speedup vs baseline: 1.3576x; 1.0019x over previous
"""Trainium2 Bass kernel for GQA attention (B=2, S=2048, D=2048, 16 q-heads,
4 kv-heads, head_dim=128, RoPE, causal) sharded over 8 NeuronCores.

Sharding: core c handles batch b = c//4 and q-head group g = c%4
(q-heads 4g..4g+3, which share kv-head g).  Each core computes a partial
output o_part[b] = sum_{its heads} attn_head @ Wo_head; the host sums the
4 partials per batch.

Schedule: fully software-pipelined over 512-row blocks.  Per block sb:
q/k/v projections (rope fused) -> attention heads 0..3 -> O-projection of
block sb-1 interleaved between heads.  All input DMAs ride the sync HWDGE
ring in exact consumption order (the ring serializes trigger-with-transfer,
so order IS the arrival schedule); rope tables are computed on the
otherwise-idle GpSimd engine so the DVE only does projection/attention
evictions.
"""

import sys

sys.path.insert(0, "/opt/trn_rl_repo")

import math

import ml_dtypes
import numpy as np

P = 128
NEG = -1.0e9
EXP_BIAS = -8.0  # exp(s - 8): cancels in softmax normalization, avoids overflow


def build_nc(S=2048, D=2048, QH=4, H=128, theta=10000.0):
    """Build the per-core Bass graph.

    Per-core problem: xt [NSB, D, SB] bf16 (x.T in 512-col blocks),
    positions [S] i32, wq [QH, P, DK, H] bf16 (pre-scaled by 1/sqrt(H),
    d-chunk-shuffled), wk/wv [P, DK, H] bf16, wo [QH, H, D] bf16
    ->  o [S, D] bf16 (partial over heads).
    """
    import concourse.bacc as bacc
    import concourse.mybir as mybir
    from concourse import tile
    from concourse.masks import make_identity

    f32 = mybir.dt.float32
    bf16 = mybir.dt.bfloat16
    i32 = mybir.dt.int32
    ADD = mybir.AluOpType.add
    MULT = mybir.AluOpType.mult
    EXP = mybir.ActivationFunctionType.Exp
    SIN = mybir.ActivationFunctionType.Sin

    assert H == P
    HH = H // 2  # 64
    DK = D // P  # d-chunks (16)
    SB = min(512, S)  # block width
    NSB = S // SB  # blocks (4)
    RB = SB // P  # 128-subtiles per block (4)
    NT = S // P  # t tiles (16)
    CS = min(256, S)  # rope table chunk width
    NCS = S // CS

    nc = bacc.Bacc(None, target_bir_lowering=False)

    xt_d = nc.declare_dram_parameter("xt", [NSB, P, DK, SB], bf16, isOutput=False)
    cos_d = nc.declare_dram_parameter("cos2", [P, S], bf16, isOutput=False)
    sin_d = nc.declare_dram_parameter("sin2s", [P, S], bf16, isOutput=False)
    wq_d = nc.declare_dram_parameter("wq", [QH, P, DK, H], bf16, isOutput=False)
    wk_d = nc.declare_dram_parameter("wk", [P, DK, H], bf16, isOutput=False)
    wv_d = nc.declare_dram_parameter("wv", [P, DK, H], bf16, isOutput=False)
    wo_d = nc.declare_dram_parameter("wo", [QH, H, D], bf16, isOutput=False)
    o_d = nc.declare_dram_parameter("o", [S, D], bf16, isOutput=True)

    from contextlib import ExitStack

    with tile.TileContext(nc) as tc, ExitStack() as es:
        # ---------------- pools ----------------
        const = es.enter_context(tc.tile_pool(name="const", bufs=1))
        persist = es.enter_context(tc.tile_pool(name="persist", bufs=1))
        small = es.enter_context(tc.tile_pool(name="small", bufs=2))
        pt_pool = es.enter_context(tc.tile_pool(name="pt", bufs=8))
        ob_pool = es.enter_context(tc.tile_pool(name="ob", bufs=2))
        at_pool = es.enter_context(tc.tile_pool(name="at", bufs=1))
        ps_sc = es.enter_context(tc.tile_pool(name="ps_sc", bufs=4, space="PSUM"))
        ps_av = es.enter_context(tc.tile_pool(name="ps_av", bufs=1, space="PSUM"))

        # ------- input DMAs: ALL on the sync ring, in consumption order ----
        # The HWDGE ring serializes trigger-with-transfer, so this order is
        # the arrival schedule: wq0 -> x block 0 (4 pieces for an early
        # start) -> wq1..3, wk, wv -> x block 1 -> wo -> x blocks 2, 3.
        wq_sb = [
            persist.tile([P, DK, H], bf16, name=f"wq{h}", tag=f"wq{h}")
            for h in range(QH)
        ]
        wk_sb = persist.tile([P, DK, H], bf16, name="wk", tag="wk")
        wv_sb = persist.tile([P, DK, H], bf16, name="wv", tag="wv")
        wo_sb = [
            persist.tile([P, D], bf16, name=f"wo{h}", tag=f"wo{h}")
            for h in range(QH)
        ]
        # block-major free layout: each x block (and each dk-group piece of
        # block 0) is one CONTIGUOUS interval of the tile, so the tile
        # framework's interval-based dependency tracking never creates false
        # cross-block waits (the [P, DK, S] layout interleaved blocks with
        # dk stripes and serialized projections behind later block DMAs)
        xT = persist.tile([P, NSB, DK, SB], bf16)

        def dma_x_block(sb, ngroups=1):
            DG = DK // ngroups
            for dk0 in range(0, DK, DG):
                nc.sync.dma_start(
                    xT[:, sb, dk0 : dk0 + DG, :],
                    xt_d[sb, :, dk0 : dk0 + DG, :],
                )

        cos2 = persist.tile([P, S], bf16)
        sin2s = persist.tile([P, S], bf16)

        # x + projection weights on the sync ring in consumption order;
        # rope tables (bf16) + wo on the scalar ring.  One ring alone cannot
        # saturate HBM (~40us slower measured), so both run concurrently.
        # block 0 arrives as interleaved (wq0 quarter, x 2-dk piece) pairs in
        # dk order, so the first projection matmul starts ~4us earlier and
        # the dk loop trickles right behind the ring
        for dk0 in range(0, DK, 4):
            nc.sync.dma_start(
                wq_sb[0][:, dk0 : dk0 + 4, :], wq_d[0, :, dk0 : dk0 + 4, :]
            )
            nc.sync.dma_start(
                xT[:, 0, dk0 : dk0 + 2, :], xt_d[0, :, dk0 : dk0 + 2, :]
            )
            nc.sync.dma_start(
                xT[:, 0, dk0 + 2 : dk0 + 4, :], xt_d[0, :, dk0 + 2 : dk0 + 4, :]
            )
        for h in range(1, QH):
            nc.sync.dma_start(wq_sb[h][:], wq_d[h])
        nc.sync.dma_start(wk_sb[:], wk_d[:])
        nc.sync.dma_start(wv_sb[:], wv_d[:])
        dma_x_block(1)
        dma_x_block(2)
        dma_x_block(3)
        nc.scalar.dma_start(cos2[:], cos_d[:])
        nc.scalar.dma_start(sin2s[:], sin_d[:])
        for h in range(QH):
            nc.scalar.dma_start(wo_sb[h][:], wo_d[h])

        # ---------------- constants (gpsimd) ----------------
        ident = const.tile([P, P], bf16)
        make_identity(nc, ident)

        exp_bias = const.tile([P, 1], f32)
        nc.gpsimd.memset(exp_bias[:], EXP_BIAS)

        # causal additive mask for the diagonal [P, P] sub-block of a
        # scoresT tile: keep (0) where y >= x, else NEG.
        mask = const.tile([P, P], f32)
        nc.gpsimd.memset(mask[:], 0.0)
        nc.gpsimd.affine_select(
            out=mask[:],
            in_=mask[:],
            compare_op=mybir.AluOpType.is_ge,
            fill=NEG,
            base=0,
            pattern=[[1, P]],
            channel_multiplier=-1,
        )

        # ---------------- PE warm-up ----------------
        # the Tensor engine needs ~3us of continuous execution to reach full
        # clock; during block 0's DMA-paced trickle each stall resets the
        # ramp.  A dummy matmul burst into the (idle until attention) AV
        # PSUM bank keeps the ramp alive across the early inter-pass gaps.
        def warm_pe(n):
            dm = ps_av.tile([P, H + 1], f32, tag="av3", name="warm", bufs=1)
            for _ in range(n):
                nc.tensor.matmul(
                    dm[:, 0:P], ident[:], ident[:], start=True, stop=True
                )

        # ---------------- projections for one block ----------------
        qT = [
            persist.tile([P, S], bf16, name=f"qT{h}", tag=f"qT{h}")
            for h in range(QH)
        ]
        kT = persist.tile([P, S], bf16)
        VW = H + 4
        vp = persist.tile([P, NT, VW], bf16)

        def proj_qk(w_sb, out_tile, sb):
            sl = slice(sb * SB, (sb + 1) * SB)
            pq = ps_sc.tile([P, SB], f32, tag="sc", name="pq")
            for dk in range(DK):
                nc.tensor.matmul(
                    pq[:],
                    w_sb[:, dk, :],
                    xT[:, sb, dk, :],
                    start=(dk == 0),
                    stop=(dk == DK - 1),
                )
            # rope: out = pq * cos2 + rot(pq) * sin2s
            tsin = small.tile([P, SB], f32, tag="tsin")
            nc.vector.tensor_tensor(
                tsin[0:HH, :], pq[HH:P, :], sin2s[0:HH, sl], MULT
            )
            nc.vector.tensor_tensor(
                tsin[HH:P, :], pq[0:HH, :], sin2s[HH:P, sl], MULT
            )
            tcos = small.tile([P, SB], f32, tag="tcos")
            nc.vector.tensor_tensor(tcos[:], pq[:], cos2[:, sl], MULT)
            nc.vector.tensor_tensor(out_tile[:, sl], tcos[:], tsin[:], ADD)

        def proj_block(sb):
            for h in range(QH):
                proj_qk(wq_sb[h], qT[h], sb)
                if sb == 0 and h < 3:
                    warm_pe(8)
            proj_qk(wk_sb, kT, sb)
            # v projection (v' with ones column), one [P, P] tile per tt
            for r in range(RB):
                tt = sb * RB + r
                pv = ps_sc.tile([P, P], f32, tag="sc", name="pv")
                for dk in range(DK):
                    nc.tensor.matmul(
                        pv[:],
                        xT[:, sb, dk, r * P : (r + 1) * P],
                        wv_sb[:, dk, :],
                        start=(dk == 0),
                        stop=(dk == DK - 1),
                    )
                nc.vector.tensor_copy(vp[:, tt, 0:H], pv[:])
                nc.gpsimd.memset(vp[:, tt, H : H + 1], 1.0)

        # ---------------- attention + fused O projection ----------------
        # O-projection of block sb-1 is interleaved between the attention
        # heads of block sb so its PSUM-evict waits don't stall the PE queue.
        def oproj_tile(sb, attnT_blk, r2, orow):
            st = RB * sb + r2
            for db in range(D // SB):
                po = ps_sc.tile([P, SB], f32, tag="sc", name="po")
                for h in range(QH):
                    nc.tensor.matmul(
                        po[:],
                        attnT_blk[h][:, r2 * P : (r2 + 1) * P],
                        wo_sb[h][:, db * SB : (db + 1) * SB],
                        start=(h == 0),
                        stop=(h == QH - 1),
                    )
                if db < 2:
                    nc.scalar.copy(orow[:, db * SB : (db + 1) * SB], po[:])
                else:
                    nc.vector.tensor_copy(orow[:, db * SB : (db + 1) * SB], po[:])
            nc.sync.dma_start(o_d[st * P : (st + 1) * P, :], orow[:])

        def prefetch_scores(sb, h, look=2):
            """Emit head h's first `look` score tiles early (before the
            preceding O-projection tile) so exp has lead time over the
            head boundary.  Must mirror emit_scores exactly, including the
            diagonal-subtile mask."""
            pre = {}
            ntt = RB * (sb + 1)
            for tt in range(min(look, ntt)):
                r = tt - RB * sb
                c0 = max(0, r) * P
                pscore = ps_sc.tile([P, SB], f32, tag="sc", name="pscore")
                nc.tensor.matmul(
                    pscore[:, c0:SB],
                    kT[:, tt * P : (tt + 1) * P],
                    qT[h][:, sb * SB + c0 : (sb + 1) * SB],
                    start=True,
                    stop=True,
                )
                if r >= 0:
                    nc.vector.tensor_tensor(
                        pscore[:, r * P : (r + 1) * P],
                        pscore[:, r * P : (r + 1) * P],
                        mask[:],
                        ADD,
                    )
                pre[tt] = pscore
            return pre

        def attention_head(sb, h, attnT, post_transpose=None, pre=None):
            # one PSUM bank per AV accumulator: start=True clears has_written
            # for the WHOLE bank, so accumulation groups can never share one
            pav = [
                ps_av.tile(
                    [P, H + 1], f32, name=f"pav{r}", tag=f"av{r}", bufs=1
                )[:]
                for r in range(RB)
            ]
            ptr2 = ps_sc.tile([P, SB], bf16, tag="sc", name="ptr2")
            ans = [None] * RB

            def finish_subtile(r2):
                rec = small.tile([P, 1], f32, tag="rec", bufs=4)
                nc.vector.reciprocal(rec[:], pav[r2][:, H : H + 1])
                an = small.tile([P, H], bf16, tag="an", bufs=4)
                nc.vector.tensor_scalar_mul(an[:], pav[r2][:, 0:H], rec[:])
                ans[r2] = an

            def emit_transpose(r2):
                nc.tensor.transpose(
                    ptr2[:, r2 * P : (r2 + 1) * P], ans[r2][:], ident[:]
                )
                sl2 = slice(r2 * P, (r2 + 1) * P)
                nc.vector.tensor_copy(attnT[h][:, sl2], ptr2[:, sl2])
                if post_transpose is not None:
                    post_transpose(r2)

            ntt = RB * (sb + 1)
            LOOK = 2  # scores lookahead: exp gets 2 tiles of lead time so
            # AV(tt) never stalls the in-order PE queue waiting on exp(tt)
            pscores = pre if pre is not None else {}

            def emit_scores(tt):
                r = tt - RB * sb
                c0 = max(0, r) * P  # exact-causal: skip below-diag subtiles
                pscore = ps_sc.tile([P, SB], f32, tag="sc", name="pscore")
                nc.tensor.matmul(
                    pscore[:, c0:SB],
                    kT[:, tt * P : (tt + 1) * P],
                    qT[h][:, sb * SB + c0 : (sb + 1) * SB],
                    start=True,
                    stop=True,
                )
                if r >= 0:
                    nc.vector.tensor_tensor(
                        pscore[:, r * P : (r + 1) * P],
                        pscore[:, r * P : (r + 1) * P],
                        mask[:],
                        ADD,
                    )
                pscores[tt] = pscore

            for tt in range(min(LOOK, ntt)):
                if tt not in pscores:
                    emit_scores(tt)
            for tt in range(ntt):
                if tt + LOOK < ntt:
                    emit_scores(tt + LOOK)
                pscore = pscores.pop(tt)
                r = tt - RB * sb
                pt = pt_pool.tile([P, SB], bf16, tag="pt")
                c0 = max(0, r) * P
                nc.scalar.activation(
                    pt[:, c0:SB], pscore[:, c0:SB], EXP, bias=exp_bias[:]
                )
                for r2 in range(max(0, r), RB):
                    q128 = RB * sb + r2
                    nc.tensor.matmul(
                        pav[r2],
                        pt[:, r2 * P : (r2 + 1) * P],
                        vp[:, tt, 0 : H + 1],
                        start=(tt == 0),
                        stop=(tt == q128),
                    )
                if r >= 0:
                    finish_subtile(r)
                if r >= 1:
                    emit_transpose(r - 1)
            emit_transpose(RB - 1)

        prev = None
        for sb in range(NSB):
            proj_block(sb)
            attnT = [
                at_pool.tile(
                    [P, SB], bf16, name=f"attnT{h}", tag=f"attnT{h}", bufs=2
                )
                for h in range(QH)
            ]
            last = sb == NSB - 1
            pre = None
            for h in range(QH):
                if last and prev is not None:
                    orow = ob_pool.tile([P, D], bf16, tag="ob")
                    oproj_tile(sb - 1, prev, h, orow)
                post = None
                if last and h == QH - 1:
                    def post(r2, _attnT=attnT):
                        orow = ob_pool.tile([P, D], bf16, tag="ob")
                        oproj_tile(NSB - 1, _attnT, r2, orow)
                attention_head(sb, h, attnT, post_transpose=post, pre=pre)
                pre = prefetch_scores(sb, h + 1) if h + 1 < QH else None
                if not last and prev is not None:
                    orow = ob_pool.tile([P, D], bf16, tag="ob")
                    oproj_tile(sb - 1, prev, h, orow)
            prev = attnT

    nc.compile()
    return nc


_NC_CACHE = {}


def _get_nc(key):
    if key not in _NC_CACHE:
        _NC_CACHE[key] = build_nc(*key)
    return _NC_CACHE[key]


def make_in_maps(x, positions, Wq, Wk, Wv, Wo, n_cores=8):
    B, S, D = x.shape
    Q, _, H = Wq.shape
    N = Wk.shape[0]
    groups = Q // N if N else 1
    gpb = n_cores // B  # head groups per batch (4)
    qh_per_core = Q // gpb
    assert qh_per_core * gpb == Q
    scale = np.float32(1.0 / math.sqrt(H))
    SB = min(512, S)
    NSB = S // SB
    DK = D // P

    def shuf_w(w):  # [D, H] f32 -> [P, DK, H] bf16 (d = dk*P + p)
        return np.ascontiguousarray(
            w.reshape(DK, P, H).transpose(1, 0, 2).astype(ml_dtypes.bfloat16)
        )

    in_maps = []
    for c in range(n_cores):
        b = c // gpb
        g = c % gpb
        qh0 = g * qh_per_core
        kvh = qh0 // groups
        # [NSB, P, DK, SB]: 4KB+ contiguous segments on BOTH DMA sides
        xt = np.ascontiguousarray(
            x[b]
            .astype(ml_dtypes.bfloat16)
            .reshape(NSB, SB, DK, P)
            .transpose(0, 3, 2, 1)
        )
        in_maps.append(
            {
                "xt": xt,
                "positions": positions,
                "wq": np.stack(
                    [
                        shuf_w(Wq[qh0 + i] * scale)
                        for i in range(qh_per_core)
                    ]
                ),
                "wk": shuf_w(Wk[kvh]),
                "wv": shuf_w(Wv[kvh]),
                "wo": np.ascontiguousarray(
                    Wo[qh0 : qh0 + qh_per_core].astype(ml_dtypes.bfloat16)
                ),
            }
        )
    return in_maps, gpb, qh_per_core


def kernel(x, positions, Wq, Wk, Wv, Wo):
    """Full inputs -> full output.  x [B,S,D] f32, positions [S] i32,
    Wq [Q,D,H], Wk/Wv [N,D,H], Wo [Q,H,D].  Returns [B,S,D] f32."""
    from concourse.bass_utils import run_bass_kernel_spmd

    x = np.ascontiguousarray(np.asarray(x, dtype=np.float32))
    positions = np.ascontiguousarray(np.asarray(positions, dtype=np.int32))
    Wq = np.asarray(Wq, dtype=np.float32)
    Wk = np.asarray(Wk, dtype=np.float32)
    Wv = np.asarray(Wv, dtype=np.float32)
    Wo = np.asarray(Wo, dtype=np.float32)

    B, S, D = x.shape
    Q, _, H = Wq.shape
    n_cores = 8
    in_maps, gpb, qh_per_core = make_in_maps(x, positions, Wq, Wk, Wv, Wo, n_cores)

    nc = _get_nc((S, D, qh_per_core, H))
    res = run_bass_kernel_spmd(nc, in_maps, core_ids=list(range(n_cores)))
    out = np.zeros((B, S, D), dtype=np.float32)
    for c in range(n_cores):
        out[c // gpb] += res.results[c]["o"].astype(np.float32)
    return out


# revision 27
# speedup vs baseline: 1.3688x; 1.0082x over previous
"""Trainium2 Bass kernel for GQA attention (B=2, S=2048, D=2048, 16 q-heads,
4 kv-heads, head_dim=128, RoPE, causal) sharded over 8 NeuronCores.

Sharding: core c handles batch b = c//4 and q-head group g = c%4
(q-heads 4g..4g+3, which share kv-head g).  Each core computes a partial
output o_part[b] = sum_{its heads} attn_head @ Wo_head; the host sums the
4 partials per batch.

Schedule: fully software-pipelined over 512-row blocks.  Per block sb:
q/k/v projections (rope fused) -> attention heads 0..3 -> O-projection of
block sb-1 interleaved between heads.  All input DMAs ride the sync HWDGE
ring in exact consumption order (the ring serializes trigger-with-transfer,
so order IS the arrival schedule); rope tables are computed on the
otherwise-idle GpSimd engine so the DVE only does projection/attention
evictions.
"""

import sys

sys.path.insert(0, "/opt/trn_rl_repo")

import math

import ml_dtypes
import numpy as np

P = 128
NEG = -1.0e9
EXP_BIAS = -8.0  # exp(s - 8): cancels in softmax normalization, avoids overflow


def build_nc(S=2048, D=2048, QH=4, H=128, theta=10000.0):
    """Build the per-core Bass graph.

    Per-core problem: xt [NSB, D, SB] bf16 (x.T in 512-col blocks),
    positions [S] i32, wq [QH, P, DK, H] bf16 (pre-scaled by 1/sqrt(H),
    d-chunk-shuffled), wk/wv [P, DK, H] bf16, wo [QH, H, D] bf16
    ->  o [S, D] bf16 (partial over heads).
    """
    import concourse.bacc as bacc
    import concourse.mybir as mybir
    from concourse import tile
    from concourse.masks import make_identity

    f32 = mybir.dt.float32
    bf16 = mybir.dt.bfloat16
    i32 = mybir.dt.int32
    ADD = mybir.AluOpType.add
    MULT = mybir.AluOpType.mult
    EXP = mybir.ActivationFunctionType.Exp
    SIN = mybir.ActivationFunctionType.Sin

    assert H == P
    HH = H // 2  # 64
    DK = D // P  # d-chunks (16)
    SB = min(512, S)  # block width
    NSB = S // SB  # blocks (4)
    RB = SB // P  # 128-subtiles per block (4)
    NT = S // P  # t tiles (16)
    CS = min(256, S)  # rope table chunk width
    NCS = S // CS

    nc = bacc.Bacc(None, target_bir_lowering=False)

    xt_d = nc.declare_dram_parameter("xt", [NSB, P, DK, SB], bf16, isOutput=False)
    cos_d = nc.declare_dram_parameter("cos2", [P, S], bf16, isOutput=False)
    sin_d = nc.declare_dram_parameter("sin2s", [P, S], bf16, isOutput=False)
    wq_d = nc.declare_dram_parameter("wq", [QH, P, DK, H], bf16, isOutput=False)
    wk_d = nc.declare_dram_parameter("wk", [P, DK, H], bf16, isOutput=False)
    wv_d = nc.declare_dram_parameter("wv", [P, DK, H], bf16, isOutput=False)
    wo_d = nc.declare_dram_parameter("wo", [QH, H, D], bf16, isOutput=False)
    o_d = nc.declare_dram_parameter("o", [S, D], bf16, isOutput=True)

    from contextlib import ExitStack

    with tile.TileContext(nc) as tc, ExitStack() as es:
        # ---------------- pools ----------------
        const = es.enter_context(tc.tile_pool(name="const", bufs=1))
        persist = es.enter_context(tc.tile_pool(name="persist", bufs=1))
        small = es.enter_context(tc.tile_pool(name="small", bufs=2))
        pt_pool = es.enter_context(tc.tile_pool(name="pt", bufs=8))
        ob_pool = es.enter_context(tc.tile_pool(name="ob", bufs=2))
        at_pool = es.enter_context(tc.tile_pool(name="at", bufs=1))
        ps_sc = es.enter_context(tc.tile_pool(name="ps_sc", bufs=4, space="PSUM"))
        ps_av = es.enter_context(tc.tile_pool(name="ps_av", bufs=1, space="PSUM"))

        # ------- input DMAs: ALL on the sync ring, in consumption order ----
        # The HWDGE ring serializes trigger-with-transfer, so this order is
        # the arrival schedule: wq0 -> x block 0 (4 pieces for an early
        # start) -> wq1..3, wk, wv -> x block 1 -> wo -> x blocks 2, 3.
        wq_sb = [
            persist.tile([P, DK, H], bf16, name=f"wq{h}", tag=f"wq{h}")
            for h in range(QH)
        ]
        wk_sb = persist.tile([P, DK, H], bf16, name="wk", tag="wk")
        wv_sb = persist.tile([P, DK, H], bf16, name="wv", tag="wv")
        wo_sb = [
            persist.tile([P, D], bf16, name=f"wo{h}", tag=f"wo{h}")
            for h in range(QH)
        ]
        # block-major free layout: each x block (and each dk-group piece of
        # block 0) is one CONTIGUOUS interval of the tile, so the tile
        # framework's interval-based dependency tracking never creates false
        # cross-block waits (the [P, DK, S] layout interleaved blocks with
        # dk stripes and serialized projections behind later block DMAs)
        xT = persist.tile([P, NSB, DK, SB], bf16)

        def dma_x_block(sb, ngroups=1):
            DG = DK // ngroups
            for dk0 in range(0, DK, DG):
                nc.sync.dma_start(
                    xT[:, sb, dk0 : dk0 + DG, :],
                    xt_d[sb, :, dk0 : dk0 + DG, :],
                )

        cos2 = persist.tile([P, S], bf16)
        sin2s = persist.tile([P, S], bf16)

        # x + projection weights on the sync ring in consumption order;
        # rope tables (bf16) + wo on the scalar ring.  One ring alone cannot
        # saturate HBM (~40us slower measured), so both run concurrently.
        nc.sync.dma_start(wq_sb[0][:], wq_d[0])
        dma_x_block(0, ngroups=4)
        for h in range(1, QH):
            nc.sync.dma_start(wq_sb[h][:], wq_d[h])
        nc.sync.dma_start(wk_sb[:], wk_d[:])
        nc.sync.dma_start(wv_sb[:], wv_d[:])
        dma_x_block(1)
        dma_x_block(2)
        dma_x_block(3)
        nc.scalar.dma_start(cos2[:], cos_d[:])
        nc.scalar.dma_start(sin2s[:], sin_d[:])
        for h in range(QH):
            nc.scalar.dma_start(wo_sb[h][:], wo_d[h])

        # ---------------- constants (gpsimd) ----------------
        ident = const.tile([P, P], bf16)
        make_identity(nc, ident)

        exp_bias = const.tile([P, 1], f32)
        nc.gpsimd.memset(exp_bias[:], EXP_BIAS)

        # causal additive mask for the diagonal [P, P] sub-block of a
        # scoresT tile: keep (0) where y >= x, else NEG.
        mask = const.tile([P, P], f32)
        nc.gpsimd.memset(mask[:], 0.0)
        nc.gpsimd.affine_select(
            out=mask[:],
            in_=mask[:],
            compare_op=mybir.AluOpType.is_ge,
            fill=NEG,
            base=0,
            pattern=[[1, P]],
            channel_multiplier=-1,
        )

        # ---------------- projections for one block ----------------
        qT = [
            persist.tile([P, S], bf16, name=f"qT{h}", tag=f"qT{h}")
            for h in range(QH)
        ]
        kT = persist.tile([P, S], bf16)
        VW = H + 4
        vp = persist.tile([P, NT, VW], bf16)

        def proj_qk(w_sb, out_tile, sb):
            sl = slice(sb * SB, (sb + 1) * SB)
            pq = ps_sc.tile([P, SB], f32, tag="sc", name="pq")
            for dk in range(DK):
                nc.tensor.matmul(
                    pq[:],
                    w_sb[:, dk, :],
                    xT[:, sb, dk, :],
                    start=(dk == 0),
                    stop=(dk == DK - 1),
                )
            # rope: out = pq * cos2 + rot(pq) * sin2s
            tsin = small.tile([P, SB], f32, tag="tsin")
            nc.vector.tensor_tensor(
                tsin[0:HH, :], pq[HH:P, :], sin2s[0:HH, sl], MULT
            )
            nc.vector.tensor_tensor(
                tsin[HH:P, :], pq[0:HH, :], sin2s[HH:P, sl], MULT
            )
            tcos = small.tile([P, SB], f32, tag="tcos")
            nc.vector.tensor_tensor(tcos[:], pq[:], cos2[:, sl], MULT)
            nc.vector.tensor_tensor(out_tile[:, sl], tcos[:], tsin[:], ADD)

        def proj_block(sb):
            for h in range(QH):
                proj_qk(wq_sb[h], qT[h], sb)
            proj_qk(wk_sb, kT, sb)
            # v projection (v' with ones column), one [P, P] tile per tt
            for r in range(RB):
                tt = sb * RB + r
                pv = ps_sc.tile([P, P], f32, tag="sc", name="pv")
                for dk in range(DK):
                    nc.tensor.matmul(
                        pv[:],
                        xT[:, sb, dk, r * P : (r + 1) * P],
                        wv_sb[:, dk, :],
                        start=(dk == 0),
                        stop=(dk == DK - 1),
                    )
                nc.vector.tensor_copy(vp[:, tt, 0:H], pv[:])
                nc.gpsimd.memset(vp[:, tt, H : H + 1], 1.0)

        # ---------------- attention + fused O projection ----------------
        # O-projection of block sb-1 is interleaved between the attention
        # heads of block sb so its PSUM-evict waits don't stall the PE queue.
        def oproj_tile(sb, attnT_blk, r2, orow):
            st = RB * sb + r2
            for db in range(D // SB):
                po = ps_sc.tile([P, SB], f32, tag="sc", name="po")
                for h in range(QH):
                    nc.tensor.matmul(
                        po[:],
                        attnT_blk[h][:, r2 * P : (r2 + 1) * P],
                        wo_sb[h][:, db * SB : (db + 1) * SB],
                        start=(h == 0),
                        stop=(h == QH - 1),
                    )
                if db < 2:
                    nc.scalar.copy(orow[:, db * SB : (db + 1) * SB], po[:])
                else:
                    nc.vector.tensor_copy(orow[:, db * SB : (db + 1) * SB], po[:])
            nc.sync.dma_start(o_d[st * P : (st + 1) * P, :], orow[:])

        def prefetch_scores(sb, h, look=2):
            """Emit head h's first `look` score tiles early (before the
            preceding O-projection tile) so exp has lead time over the
            head boundary.  Must mirror emit_scores exactly, including the
            diagonal-subtile mask."""
            pre = {}
            ntt = RB * (sb + 1)
            for tt in range(min(look, ntt)):
                r = tt - RB * sb
                c0 = max(0, r) * P
                pscore = ps_sc.tile([P, SB], f32, tag="sc", name="pscore")
                nc.tensor.matmul(
                    pscore[:, c0:SB],
                    kT[:, tt * P : (tt + 1) * P],
                    qT[h][:, sb * SB + c0 : (sb + 1) * SB],
                    start=True,
                    stop=True,
                )
                if r >= 0:
                    nc.vector.tensor_tensor(
                        pscore[:, r * P : (r + 1) * P],
                        pscore[:, r * P : (r + 1) * P],
                        mask[:],
                        ADD,
                    )
                pre[tt] = pscore
            return pre

        def attention_head(sb, h, attnT, post_transpose=None, pre=None):
            # one PSUM bank per AV accumulator: start=True clears has_written
            # for the WHOLE bank, so accumulation groups can never share one
            pav = [
                ps_av.tile(
                    [P, H + 1], f32, name=f"pav{r}", tag=f"av{r}", bufs=1
                )[:]
                for r in range(RB)
            ]
            ptr2 = ps_sc.tile([P, SB], bf16, tag="sc", name="ptr2")
            ans = [None] * RB

            def finish_subtile(r2):
                rec = small.tile([P, 1], f32, tag="rec", bufs=4)
                nc.vector.reciprocal(rec[:], pav[r2][:, H : H + 1])
                an = small.tile([P, H], bf16, tag="an", bufs=4)
                nc.vector.tensor_scalar_mul(an[:], pav[r2][:, 0:H], rec[:])
                ans[r2] = an

            def emit_transpose(r2):
                nc.tensor.transpose(
                    ptr2[:, r2 * P : (r2 + 1) * P], ans[r2][:], ident[:]
                )
                sl2 = slice(r2 * P, (r2 + 1) * P)
                nc.vector.tensor_copy(attnT[h][:, sl2], ptr2[:, sl2])
                if post_transpose is not None:
                    post_transpose(r2)

            ntt = RB * (sb + 1)
            LOOK = 2  # scores lookahead: exp gets 2 tiles of lead time so
            # AV(tt) never stalls the in-order PE queue waiting on exp(tt)
            pscores = pre if pre is not None else {}

            def emit_scores(tt):
                r = tt - RB * sb
                c0 = max(0, r) * P  # exact-causal: skip below-diag subtiles
                pscore = ps_sc.tile([P, SB], f32, tag="sc", name="pscore")
                nc.tensor.matmul(
                    pscore[:, c0:SB],
                    kT[:, tt * P : (tt + 1) * P],
                    qT[h][:, sb * SB + c0 : (sb + 1) * SB],
                    start=True,
                    stop=True,
                )
                if r >= 0:
                    nc.vector.tensor_tensor(
                        pscore[:, r * P : (r + 1) * P],
                        pscore[:, r * P : (r + 1) * P],
                        mask[:],
                        ADD,
                    )
                pscores[tt] = pscore

            for tt in range(min(LOOK, ntt)):
                if tt not in pscores:
                    emit_scores(tt)
            for tt in range(ntt):
                if tt + LOOK < ntt:
                    emit_scores(tt + LOOK)
                pscore = pscores.pop(tt)
                r = tt - RB * sb
                pt = pt_pool.tile([P, SB], bf16, tag="pt")
                c0 = max(0, r) * P
                nc.scalar.activation(
                    pt[:, c0:SB], pscore[:, c0:SB], EXP, bias=exp_bias[:]
                )
                for r2 in range(max(0, r), RB):
                    q128 = RB * sb + r2
                    nc.tensor.matmul(
                        pav[r2],
                        pt[:, r2 * P : (r2 + 1) * P],
                        vp[:, tt, 0 : H + 1],
                        start=(tt == 0),
                        stop=(tt == q128),
                    )
                if r >= 0:
                    finish_subtile(r)
                if r >= 1:
                    emit_transpose(r - 1)
            emit_transpose(RB - 1)

        prev = None
        for sb in range(NSB):
            proj_block(sb)
            attnT = [
                at_pool.tile(
                    [P, SB], bf16, name=f"attnT{h}", tag=f"attnT{h}", bufs=2
                )
                for h in range(QH)
            ]
            last = sb == NSB - 1
            pre = None
            for h in range(QH):
                if last and prev is not None:
                    orow = ob_pool.tile([P, D], bf16, tag="ob")
                    oproj_tile(sb - 1, prev, h, orow)
                post = None
                if last and h == QH - 1:
                    def post(r2, _attnT=attnT):
                        orow = ob_pool.tile([P, D], bf16, tag="ob")
                        oproj_tile(NSB - 1, _attnT, r2, orow)
                attention_head(sb, h, attnT, post_transpose=post, pre=pre)
                pre = prefetch_scores(sb, h + 1) if h + 1 < QH else None
                if not last and prev is not None:
                    orow = ob_pool.tile([P, D], bf16, tag="ob")
                    oproj_tile(sb - 1, prev, h, orow)
            prev = attnT

    nc.compile()
    return nc


_NC_CACHE = {}


def _get_nc(key):
    if key not in _NC_CACHE:
        _NC_CACHE[key] = build_nc(*key)
    return _NC_CACHE[key]


def make_in_maps(x, positions, Wq, Wk, Wv, Wo, n_cores=8):
    B, S, D = x.shape
    Q, _, H = Wq.shape
    N = Wk.shape[0]
    groups = Q // N if N else 1
    gpb = n_cores // B  # head groups per batch (4)
    qh_per_core = Q // gpb
    assert qh_per_core * gpb == Q
    scale = np.float32(1.0 / math.sqrt(H))
    SB = min(512, S)
    NSB = S // SB
    DK = D // P

    def shuf_w(w):  # [D, H] f32 -> [P, DK, H] bf16 (d = dk*P + p)
        return np.ascontiguousarray(
            w.reshape(DK, P, H).transpose(1, 0, 2).astype(ml_dtypes.bfloat16)
        )

    in_maps = []
    for c in range(n_cores):
        b = c // gpb
        g = c % gpb
        qh0 = g * qh_per_core
        kvh = qh0 // groups
        # [NSB, P, DK, SB]: 4KB+ contiguous segments on BOTH DMA sides
        xt = np.ascontiguousarray(
            x[b]
            .astype(ml_dtypes.bfloat16)
            .reshape(NSB, SB, DK, P)
            .transpose(0, 3, 2, 1)
        )
        in_maps.append(
            {
                "xt": xt,
                "positions": positions,
                "wq": np.stack(
                    [
                        shuf_w(Wq[qh0 + i] * scale)
                        for i in range(qh_per_core)
                    ]
                ),
                "wk": shuf_w(Wk[kvh]),
                "wv": shuf_w(Wv[kvh]),
                "wo": np.ascontiguousarray(
                    Wo[qh0 : qh0 + qh_per_core].astype(ml_dtypes.bfloat16)
                ),
            }
        )
    return in_maps, gpb, qh_per_core


def kernel(x, positions, Wq, Wk, Wv, Wo):
    """Full inputs -> full output.  x [B,S,D] f32, positions [S] i32,
    Wq [Q,D,H], Wk/Wv [N,D,H], Wo [Q,H,D].  Returns [B,S,D] f32."""
    from concourse.bass_utils import run_bass_kernel_spmd

    x = np.ascontiguousarray(np.asarray(x, dtype=np.float32))
    positions = np.ascontiguousarray(np.asarray(positions, dtype=np.int32))
    Wq = np.asarray(Wq, dtype=np.float32)
    Wk = np.asarray(Wk, dtype=np.float32)
    Wv = np.asarray(Wv, dtype=np.float32)
    Wo = np.asarray(Wo, dtype=np.float32)

    B, S, D = x.shape
    Q, _, H = Wq.shape
    n_cores = 8
    in_maps, gpb, qh_per_core = make_in_maps(x, positions, Wq, Wk, Wv, Wo, n_cores)

    nc = _get_nc((S, D, qh_per_core, H))
    res = run_bass_kernel_spmd(nc, in_maps, core_ids=list(range(n_cores)))
    out = np.zeros((B, S, D), dtype=np.float32)
    for c in range(n_cores):
        out[c // gpb] += res.results[c]["o"].astype(np.float32)
    return out
